# revision 6
# baseline (speedup 1.0000x reference)
"""CrossViewSelfAttentionFusion Trainium2 kernel (8-core SPMD).

Sharding: tokens are sharded by L-position — core R owns positions
l in [R*128, (R+1)*128) of ALL 4 views (512 tokens, ordered n-major).
Encoder self-attention is full attention over all 4096 tokens, so K/V
are AllGathered (bf16) per layer; token order inside the gathered
buffer is irrelevant. The cross-view tail at position l needs exactly
the 4 view-tokens at l, which the core already owns -> tail is fully
local and the output is assembled on the host.

Layout: activations feature-major (x^T: [C partitions, tokens free]).
"""
import math
import numpy as np
from contextlib import ExitStack

import ml_dtypes
import concourse.bass as bass
import concourse.bacc as bacc
import concourse.tile as tile
from concourse import mybir
from concourse.bass_utils import run_bass_kernel_spmd

F32 = mybir.dt.float32
BF16 = mybir.dt.bfloat16
AF = mybir.ActivationFunctionType
ALU = mybir.AluOpType

N, L, C, NH, NL = 4, 1024, 256, 8, 3
DFF = 4 * C
S = N * L            # 4096 tokens
T = S // 8           # 512 tokens per core
LT = L // 8          # 128 L-positions per core
DH = C // NH         # 32
QSCALE = 1.0 / math.sqrt(DH)
EPS = 1e-5
NCH = 8              # gathered key chunks (one per rank)
KT_PER_CH = 4

MMDT = BF16


def _ln_feature_major(nc, pstats, tmp, x_tiles, g_cols, b_cols, out_tiles, ones_col,
                      ones_row, width, tag, eps1=None):
    """LayerNorm over the 256-channel partition axis, feature-major tiles.
    x_tiles: 2 SBUF APs [128, width] (fp32). out_tiles: 2 SBUF APs (MMDT)."""
    sq = tmp.tile([128, 2, width], MMDT, name=f"lnsq_{tag}", tag="lnsq", bufs=1)
    xb = tmp.tile([128, 2, width], MMDT, name=f"lnxb_{tag}", tag="lnxb", bufs=1)
    for i in range(2):
        nc.vector.tensor_mul(sq[:, i, :], x_tiles[i], x_tiles[i])
        nc.vector.tensor_copy(xb[:, i, :], x_tiles[i])
    st0 = pstats.tile([1, width], F32, name=f"lnst0_{tag}", tag="lnst0", bufs=1)
    st1 = pstats.tile([1, width], F32, name=f"lnst1_{tag}", tag="lnst1", bufs=1)
    nc.tensor.matmul(st0, ones_col, xb[:, 0, :], start=True, stop=False)
    nc.tensor.matmul(st0, ones_col, xb[:, 1, :], start=False, stop=True)
    nc.tensor.matmul(st1, ones_col, sq[:, 0, :], start=True, stop=False)
    nc.tensor.matmul(st1, ones_col, sq[:, 1, :], start=False, stop=True)
    mean = tmp.tile([1, width], F32, name=f"lnmean_{tag}", tag="lnmean", bufs=1)
    var = tmp.tile([1, width], F32, name=f"lnvar_{tag}", tag="lnvar", bufs=1)
    rstd = tmp.tile([1, width], F32, name=f"lnrstd_{tag}", tag="lnrstd", bufs=1)
    nmr = tmp.tile([1, width], F32, name=f"lnnmr_{tag}", tag="lnnmr", bufs=1)
    nc.vector.tensor_scalar_mul(mean, st0, 1.0 / C)
    nc.vector.tensor_mul(var, mean, mean)
    nc.vector.tensor_scalar_mul(nmr, st1, 1.0 / C)
    nc.vector.tensor_sub(var, nmr, var)
    # rsqrt(var + eps) = exp(-0.5 * ln(var + eps)); Ln/Exp share one table
    nc.scalar.activation(out=rstd, in_=var, func=AF.Ln, bias=eps1, scale=1.0)
    nc.scalar.activation(out=rstd, in_=rstd, func=AF.Exp, bias=0.0, scale=-0.5)
    nc.vector.tensor_scalar_mul(nmr, mean, -1.0)
    nc.vector.tensor_mul(nmr, nmr, rstd)
    smb0 = tmp.tile([1, width], MMDT, name=f"lnsmb0_{tag}", tag="lnsmb0", bufs=1)
    smb1 = tmp.tile([1, width], MMDT, name=f"lnsmb1_{tag}", tag="lnsmb1", bufs=1)
    nc.vector.tensor_copy(smb0, rstd)
    nc.vector.tensor_copy(smb1, nmr)
    bc = pstats.tile([128, 2, width], F32, name=f"lnbc_{tag}", tag="lnbc", bufs=1)
    nc.tensor.matmul(bc[:, 0, :], ones_row, smb0, start=True, stop=True)
    nc.tensor.matmul(bc[:, 1, :], ones_row, smb1, start=True, stop=True)
    t = tmp.tile([128, 2, width], F32, name=f"lnt_{tag}", tag="lnt", bufs=1)
    for i in range(2):
        nc.vector.tensor_mul(t[:, i, :], x_tiles[i], bc[:, 0, :])
        nc.vector.tensor_add(t[:, i, :], t[:, i, :], bc[:, 1, :])
        nc.vector.tensor_scalar(out=out_tiles[i], in0=t[:, i, :], scalar1=g_cols[i],
                                scalar2=b_cols[i], op0=ALU.mult, op1=ALU.add)


def build(residual_weight: float):
    nc = bacc.Bacc("TRN2", target_bir_lowering=False, debug=False, num_devices=8)

    def inp(name, shape, dt=F32):
        return nc.dram_tensor(name, shape, dt, kind="ExternalInput")

    fT = inp("fT", (256, T))                 # features, token cols (n-major)
    posvT = inp("posvT", (256, T))           # pos + view emb, same order
    wqkvT = inp("wqkvT", (NL, 256, 3 * C), BF16)
    bqkv_c = inp("bqkv_c", (NL, 128, 6))
    bv_row = inp("bv_row", (NL, 256))
    woT = inp("woT", (NL, 256, 256), BF16)
    bo_c = inp("bo_c", (NL, 128, 2))
    w1T = inp("w1T", (NL, 256, DFF), BF16)
    b1_c = inp("b1_c", (NL, 128, 8))
    w2T = inp("w2T", (NL, DFF, 256), BF16)
    b2_c = inp("b2_c", (NL, 128, 2))
    ln1g = inp("ln1g", (NL, 128, 2))
    ln1b = inp("ln1b", (NL, 128, 2))
    ln2g = inp("ln2g", (NL, 128, 2))
    ln2b = inp("ln2b", (NL, 128, 2))
    flng = inp("flng", (128, 2))
    flnb = inp("flnb", (128, 2))
    gqT = inp("gqT", (256, LT), BF16)        # this core's global-query slice
    wvaT = inp("wvaT", (256, 3 * C), BF16)
    bva_c = inp("bva_c", (128, 6))
    bvva_row = inp("bvva_row", (1, 256))
    wovaT = inp("wovaT", (256, 256), BF16)
    bova_c = inp("bova_c", (128, 2))
    op1T = inp("op1T", (256, 256), BF16)
    bop1_c = inp("bop1_c", (128, 2))
    oplng = inp("oplng", (128, 2))
    oplnb = inp("oplnb", (128, 2))
    op2T = inp("op2T", (256, 256), BF16)
    bop2_c = inp("bop2_c", (128, 2))
    fmeanT = inp("fmeanT", (256, LT))        # (1/N)*sum_n features[n], L-slice
    bd = inp("bd", (128, 128), BF16)         # block-diagonal 32x32 ones

    o_t = nc.dram_tensor("o_t", (256, LT), F32, kind="ExternalOutput")

    kag_ins = [nc.dram_tensor(f"kag_in{i}", (256, T), BF16, kind="Internal")
               for i in range(2)]
    kag_outs = [nc.dram_tensor(f"kag_out{i}", (8, 256, T), BF16, kind="Internal",
                               addr_space="Shared") for i in range(2)]
    vag_ins = [nc.dram_tensor(f"vag_in{i}", (T, 256), BF16, kind="Internal")
               for i in range(2)]
    vag_outs = [nc.dram_tensor(f"vag_out{i}", (8, T, 256), BF16, kind="Internal",
                               addr_space="Shared") for i in range(2)]
    RG = [list(range(8))]

    with tile.TileContext(nc) as tc, ExitStack() as ctx:
        persist = ctx.enter_context(tc.tile_pool(name="persist", bufs=1))
        enc_ctx = ExitStack()
        wpool = enc_ctx.enter_context(tc.tile_pool(name="wpool", bufs=2))
        act = enc_ctx.enter_context(tc.tile_pool(name="act", bufs=2))
        ppool = enc_ctx.enter_context(tc.tile_pool(name="ppool", bufs=4))
        tmp = enc_ctx.enter_context(tc.tile_pool(name="tmp", bufs=2))

        ones_col = persist.tile([128, 1], MMDT)
        nc.vector.memset(ones_col, 1.0)
        ones_row = persist.tile([1, 128], MMDT)
        nc.vector.memset(ones_row, 1.0)
        ones32 = persist.tile([128, 32], MMDT)
        nc.vector.memset(ones32, 1.0)
        eps1 = persist.tile([1, 1], F32)
        nc.vector.memset(eps1, EPS)

        # ------- embed: x = f + (pos + view) -------
        ft_sb = tmp.tile([128, 2, T], F32)
        nc.sync.dma_start(out=ft_sb, in_=fT.rearrange("(a p) t -> p a t", p=128))
        pos_sb = tmp.tile([128, 2, T], F32)
        nc.sync.dma_start(out=pos_sb, in_=posvT.rearrange("(a p) t -> p a t", p=128))
        x_sb = persist.tile([128, 2, T], F32)
        for i in range(2):
            nc.vector.tensor_add(x_sb[:, i, :], pos_sb[:, i, :], ft_sb[:, i, :])

        # ================= encoder layers =================
        for layer in range(NL):
            # --- weights (bf16 in DRAM already) ---
            wqkv_sb = wpool.tile([128, 2, 3 * C], MMDT, name="wqkv_sb", tag="wqkv")
            nc.gpsimd.dma_start(out=wqkv_sb,
                                in_=wqkvT[layer].rearrange("(a p) o -> p a o", p=128))
            wo_sb = wpool.tile([128, 2, 256], MMDT, name="wo_sb", tag="wo")
            nc.gpsimd.dma_start(out=wo_sb,
                                in_=woT[layer].rearrange("(a p) o -> p a o", p=128))
            w1_sb = wpool.tile([128, 2, DFF], MMDT, name="w1_sb", tag="w1")
            nc.gpsimd.dma_start(out=w1_sb,
                                in_=w1T[layer].rearrange("(a p) o -> p a o", p=128))
            w2_sb = wpool.tile([128, 8, 256], MMDT, name="w2_sb", tag="w2")
            nc.gpsimd.dma_start(out=w2_sb,
                                in_=w2T[layer].rearrange("(a p) o -> p a o", p=128))
            bq_sb = wpool.tile([128, 6], F32, name="bq_sb", tag="bq")
            nc.sync.dma_start(out=bq_sb, in_=bqkv_c[layer])
            bvr_sb = wpool.tile([1, 256], MMDT, name="bvr_sb", tag="bvr")
            nc.gpsimd.dma_start(out=bvr_sb, in_=bv_row[layer][None, :])
            bo_sb = wpool.tile([128, 2], F32, name="bo_sb", tag="bo")
            nc.sync.dma_start(out=bo_sb, in_=bo_c[layer])
            b1_sb = wpool.tile([128, 8], F32, name="b1_sb", tag="b1")
            nc.sync.dma_start(out=b1_sb, in_=b1_c[layer])
            b2_sb = wpool.tile([128, 2], F32, name="b2_sb", tag="b2")
            nc.sync.dma_start(out=b2_sb, in_=b2_c[layer])
            l1g_sb = wpool.tile([128, 2], F32, name="l1g_sb", tag="l1g")
            nc.sync.dma_start(out=l1g_sb, in_=ln1g[layer])
            l1b_sb = wpool.tile([128, 2], F32, name="l1b_sb", tag="l1b")
            nc.sync.dma_start(out=l1b_sb, in_=ln1b[layer])
            l2g_sb = wpool.tile([128, 2], F32, name="l2g_sb", tag="l2g")
            nc.sync.dma_start(out=l2g_sb, in_=ln2g[layer])
            l2b_sb = wpool.tile([128, 2], F32, name="l2b_sb", tag="l2b")
            nc.sync.dma_start(out=l2b_sb, in_=ln2b[layer])

            # --- LN1 ---
            ln_sb = act.tile([128, 2, T], MMDT, name="ln_sb", tag="ln")
            with tc.tile_pool(name="pstats", bufs=1, space="PSUM") as pstats:
                _ln_feature_major(nc, pstats, tmp,
                                  [x_sb[:, 0, :], x_sb[:, 1, :]],
                                  [l1g_sb[:, 0:1], l1g_sb[:, 1:2]],
                                  [l1b_sb[:, 0:1], l1b_sb[:, 1:2]],
                                  [ln_sb[:, 0, :], ln_sb[:, 1, :]],
                                  ones_col, ones_row, T, f"l{layer}a", eps1)

            # --- QKV (bias/scale on DVE, not Scalar) ---
            q_sb = act.tile([128, 2, T], MMDT, name="q_sb", tag="q")
            k_sb = act.tile([128, 2, T], MMDT, name="k_sb", tag="k")
            v_sb = act.tile([128, 4, 256], MMDT, name="v_sb", tag="v")
            bqs_sb = tmp.tile([128, 2], F32, name="bqs_sb", tag="bqs")
            nc.vector.tensor_scalar_mul(bqs_sb, bq_sb[:, 0:2], QSCALE)
            with tc.tile_pool(name="pqkv", bufs=2, space="PSUM") as pqkv:
                for co in range(2):
                    kp = pqkv.tile([128, T], F32, name="kp", tag="qp", bufs=2)
                    for ci in range(2):
                        nc.tensor.matmul(
                            kp, wqkv_sb[:, ci, 256 + co * 128:256 + (co + 1) * 128],
                            ln_sb[:, ci, :], start=(ci == 0), stop=(ci == 1))
                    nc.vector.tensor_scalar_add(k_sb[:, co, :], kp,
                                                bq_sb[:, 2 + co:3 + co])
                # K AllGather as soon as K is ready (overlaps V/Q compute).
                # Buffers alternate by layer parity to avoid WAR stalls on
                # the previous layer's chunk reads.
                kag_in = kag_ins[layer % 2]
                kag_out = kag_outs[layer % 2]
                vag_in = vag_ins[layer % 2]
                vag_out = vag_outs[layer % 2]
                kag_v = kag_in.rearrange("(a p) t -> p a t", p=128)
                for i in range(2):
                    nc.sync.dma_start(out=kag_v[:, i, :], in_=k_sb[:, i, :])
                nc.gpsimd.collective_compute(
                    "AllGather", ALU.bypass, replica_groups=RG,
                    ins=[kag_in[:].opt()], outs=[kag_out[:].opt()])

                bvb = pqkv.tile([128, 256], F32, name="bvb", tag="bvb")
                nc.tensor.matmul(bvb, ones_row, bvr_sb, start=True, stop=True)
                bvb_sb = tmp.tile([128, 256], F32, name="bvb_sb", tag="bvb_sb")
                nc.vector.tensor_copy(bvb_sb, bvb)
                for tt in range(4):
                    vp = pqkv.tile([128, 256], F32, name="vp", tag="vp", bufs=2)
                    for ci in range(2):
                        nc.tensor.matmul(
                            vp, ln_sb[:, ci, tt * 128:(tt + 1) * 128],
                            wqkv_sb[:, ci, 512:768],
                            start=(ci == 0), stop=(ci == 1))
                    nc.vector.tensor_add(v_sb[:, tt, :], vp, bvb_sb)
                vag_v = vag_in.rearrange("(t p) c -> p t c", p=128)
                for tt in range(4):
                    nc.sync.dma_start(out=vag_v[:, tt, :], in_=v_sb[:, tt, :])
                nc.gpsimd.collective_compute(
                    "AllGather", ALU.bypass, replica_groups=RG,
                    ins=[vag_in[:].opt()], outs=[vag_out[:].opt()])

                for co in range(2):
                    qp = pqkv.tile([128, T], F32, name="qp", tag="qp", bufs=2)
                    for ci in range(2):
                        nc.tensor.matmul(
                            qp, wqkv_sb[:, ci, co * 128:(co + 1) * 128],
                            ln_sb[:, ci, :], start=(ci == 0), stop=(ci == 1))
                    # q = qp*QSCALE + bq*QSCALE  (DVE, per-partition scalars)
                    nc.vector.tensor_scalar(out=q_sb[:, co, :], in0=qp,
                                            scalar1=QSCALE,
                                            scalar2=bqs_sb[:, co:co + 1],
                                            op0=ALU.mult, op1=ALU.add)

            # --- attention ---
            # per (ch, kt, hg): two head-pairs; each pair = 2 score matmuls
            # into one [128, 2, 512] PSUM tile, ONE exp over 1024 free elems,
            # then 2 AV + 2 den matmuls. Score PSUM double-buffered so the
            # PE never waits on the Scalar exp.
            oT_sb = act.tile([128, 2, T], MMDT, name="oT_sb", tag="oT")
            with tc.tile_pool(name="psc", bufs=2, space="PSUM") as psc, \
                 tc.tile_pool(name="pacc", bufs=1, space="PSUM") as pacc:
                av_ps = [pacc.tile([128, T], F32, name=f"av_ps{g}", tag=f"av{g}")
                         for g in range(2)]
                den_ps = [pacc.tile([128, T], F32, name=f"den_ps{g}", tag=f"den{g}")
                          for g in range(2)]
                pending = None  # (pe_tile, hg, g_kt) awaiting AV/den
                for ch in range(NCH):
                    kc = tmp.tile([128, 2, T], MMDT, name="kc", tag="kc", bufs=3)
                    nc.gpsimd.dma_start(
                        out=kc, in_=kag_out[ch].rearrange("(a p) t -> p a t", p=128))
                    vc = tmp.tile([128, 4, 256], MMDT, name="vc", tag="vc", bufs=3)
                    nc.gpsimd.dma_start(
                        out=vc, in_=vag_out[ch].rearrange("(t p) c -> p t c", p=128))
                    for kt in range(KT_PER_CH):
                        g_kt = ch * KT_PER_CH + kt
                        for hg in range(2):
                            for pr in range(2):
                                sp = psc.tile([128, 2, T], F32, name="sp",
                                              tag="sp", bufs=2)
                                for j in range(2):
                                    hh = pr * 2 + j
                                    nc.tensor.matmul(
                                        sp[:, j, :],
                                        kc[hh * 32:(hh + 1) * 32, hg,
                                           kt * 128:(kt + 1) * 128],
                                        q_sb[hh * 32:(hh + 1) * 32, hg, :],
                                        start=True, stop=True,
                                        tile_position=(hh * 32, 0))
                                pe = ppool.tile([128, 2, T], MMDT, name="pe",
                                                tag="pe", bufs=4)
                                nc.scalar.activation(out=pe, in_=sp, func=AF.Exp,
                                                     bias=0.0, scale=1.0)
                                if pending is not None:
                                    _emit_avden(nc, pending, av_ps, den_ps,
                                                ones32)
                                pending = (pe, hg, pr, g_kt, vc, kt)
                _emit_avden(nc, pending, av_ps, den_ps, ones32)
                pending = None
                for hg in range(2):
                    rec = tmp.tile([128, T], F32, name="rec", tag="rec", bufs=2)
                    nc.vector.reciprocal_approx_fast(rec, den_ps[hg])
                    nc.vector.tensor_mul(oT_sb[:, hg, :], av_ps[hg], rec)

            # --- out-proj + residual ---
            with tc.tile_pool(name="pproj", bufs=1, space="PSUM") as pproj:
                for co in range(2):
                    app = pproj.tile([128, T], F32, name="app", tag="app", bufs=2)
                    for ci in range(2):
                        nc.tensor.matmul(
                            app, wo_sb[:, ci, co * 128:(co + 1) * 128],
                            oT_sb[:, ci, :], start=(ci == 0), stop=(ci == 1))
                    tres = tmp.tile([128, T], F32, name="tres", tag="tres", bufs=2)
                    nc.vector.tensor_scalar_add(tres, app, bo_sb[:, co:co + 1])
                    nc.vector.tensor_add(x_sb[:, co, :], x_sb[:, co, :], tres)

            # --- LN2 + FFN ---
            ln2_sb = act.tile([128, 2, T], MMDT, name="ln2_sb", tag="ln")
            with tc.tile_pool(name="pstats2", bufs=1, space="PSUM") as pstats2:
                _ln_feature_major(nc, pstats2, tmp,
                                  [x_sb[:, 0, :], x_sb[:, 1, :]],
                                  [l2g_sb[:, 0:1], l2g_sb[:, 1:2]],
                                  [l2b_sb[:, 0:1], l2b_sb[:, 1:2]],
                                  [ln2_sb[:, 0, :], ln2_sb[:, 1, :]],
                                  ones_col, ones_row, T, f"l{layer}b", eps1)
            h_sb = act.tile([128, 8, T], MMDT, name="h_sb", tag="h")
            with tc.tile_pool(name="pffn", bufs=1, space="PSUM") as pffn:
                for fo in range(8):
                    hp = pffn.tile([128, T], F32, name="hp", tag="hp", bufs=4)
                    for ci in range(2):
                        nc.tensor.matmul(
                            hp, w1_sb[:, ci, fo * 128:(fo + 1) * 128],
                            ln2_sb[:, ci, :], start=(ci == 0), stop=(ci == 1))
                    nc.scalar.activation(out=h_sb[:, fo, :], in_=hp, func=AF.Gelu,
                                         bias=b1_sb[:, fo:fo + 1], scale=1.0)
                for co in range(2):
                    fp = pffn.tile([128, T], F32, name="fp", tag="fp", bufs=2)
                    for fo in range(8):
                        nc.tensor.matmul(
                            fp, w2_sb[:, fo, co * 128:(co + 1) * 128],
                            h_sb[:, fo, :], start=(fo == 0), stop=(fo == 7))
                    tres2 = tmp.tile([128, T], F32, name="tres2", tag="tres", bufs=2)
                    nc.vector.tensor_scalar_add(tres2, fp, b2_sb[:, co:co + 1])
                    nc.vector.tensor_add(x_sb[:, co, :], x_sb[:, co, :], tres2)

        # ================= final LN (local) =================
        flg_sb = persist.tile([128, 2], F32)
        nc.sync.dma_start(out=flg_sb, in_=flng[:])
        flb_sb = persist.tile([128, 2], F32)
        nc.sync.dma_start(out=flb_sb, in_=flnb[:])
        fl_sb = act.tile([128, 2, T], MMDT, name="fl_sb", tag="ln")
        with tc.tile_pool(name="pstats3", bufs=1, space="PSUM") as pstats3:
            _ln_feature_major(nc, pstats3, tmp,
                              [x_sb[:, 0, :], x_sb[:, 1, :]],
                              [flg_sb[:, 0:1], flg_sb[:, 1:2]],
                              [flb_sb[:, 0:1], flb_sb[:, 1:2]],
                              [fl_sb[:, 0, :], fl_sb[:, 1, :]],
                              ones_col, ones_row, T, "fl", eps1)
        enc_ctx.close()

        # ============ cross-view tail (local: this core's LT positions) ====
        # fl_sb token order is (n, i): token n*LT + i  <-> view n, position i.
        tail = ctx.enter_context(tc.tile_pool(name="tail", bufs=1))
        ttmp = ctx.enter_context(tc.tile_pool(name="ttmp", bufs=2))
        gq_sb = tail.tile([128, 2, LT], MMDT)
        nc.gpsimd.dma_start(out=gq_sb, in_=gqT.rearrange("(a p) t -> p a t", p=128))
        wva_sb = tail.tile([128, 2, 3 * C], MMDT)
        nc.gpsimd.dma_start(out=wva_sb, in_=wvaT.rearrange("(a p) o -> p a o", p=128))
        bva_sb = tail.tile([128, 6], F32)
        nc.sync.dma_start(out=bva_sb, in_=bva_c[:])
        bvva_sb = tail.tile([1, 256], MMDT)
        nc.gpsimd.dma_start(out=bvva_sb, in_=bvva_row[:])
        wova_sb = tail.tile([128, 2, 256], MMDT)
        nc.gpsimd.dma_start(out=wova_sb, in_=wovaT.rearrange("(a p) o -> p a o", p=128))
        bova_sb = tail.tile([128, 2], F32)
        nc.sync.dma_start(out=bova_sb, in_=bova_c[:])
        op1_sb = tail.tile([128, 2, 256], MMDT)
        nc.gpsimd.dma_start(out=op1_sb, in_=op1T.rearrange("(a p) o -> p a o", p=128))
        bop1_sb = tail.tile([128, 2], F32)
        nc.sync.dma_start(out=bop1_sb, in_=bop1_c[:])
        olg_sb = tail.tile([128, 2], F32)
        nc.sync.dma_start(out=olg_sb, in_=oplng[:])
        olb_sb = tail.tile([128, 2], F32)
        nc.sync.dma_start(out=olb_sb, in_=oplnb[:])
        op2_sb = tail.tile([128, 2, 256], MMDT)
        nc.gpsimd.dma_start(out=op2_sb, in_=op2T.rearrange("(a p) o -> p a o", p=128))
        bop2_sb = tail.tile([128, 2], F32)
        nc.sync.dma_start(out=bop2_sb, in_=bop2_c[:])
        fm_sb = tail.tile([128, 2, LT], F32)
        nc.sync.dma_start(out=fm_sb, in_=fmeanT.rearrange("(a p) t -> p a t", p=128))
        bd_sb = tail.tile([128, 128], MMDT)
        nc.gpsimd.dma_start(out=bd_sb, in_=bd[:])

        # qv/kv/vv projections (feature-major, width LT per view)
        qv_sb = tail.tile([128, 2, LT], MMDT)
        kv_sb = tail.tile([128, 2, N, LT], MMDT)
        vv_sb = tail.tile([128, 2, N, LT], MMDT)
        bqs2 = ttmp.tile([128, 2], F32, name="bqs2", tag="bqs2", bufs=1)
        nc.vector.tensor_scalar_mul(bqs2, bva_sb[:, 0:2], QSCALE)
        with tc.tile_pool(name="ptail", bufs=1, space="PSUM") as ptail:
            for co in range(2):
                qp2 = ptail.tile([128, LT], F32, name="qp2", tag="tp", bufs=2)
                for ci in range(2):
                    nc.tensor.matmul(
                        qp2, wva_sb[:, ci, co * 128:(co + 1) * 128],
                        gq_sb[:, ci, :], start=(ci == 0), stop=(ci == 1))
                nc.vector.tensor_scalar(out=qv_sb[:, co, :], in0=qp2,
                                        scalar1=QSCALE, scalar2=bqs2[:, co:co + 1],
                                        op0=ALU.mult, op1=ALU.add)
                for n in range(N):
                    tok = fl_sb[:, :, n * LT:(n + 1) * LT]
                    kp2 = ptail.tile([128, LT], F32, name="kp2", tag="tp", bufs=2)
                    for ci in range(2):
                        nc.tensor.matmul(
                            kp2, wva_sb[:, ci, 256 + co * 128:256 + (co + 1) * 128],
                            tok[:, ci, :], start=(ci == 0), stop=(ci == 1))
                    nc.vector.tensor_scalar_add(kv_sb[:, co, n, :], kp2,
                                                bva_sb[:, 2 + co:3 + co])
                    vp2 = ptail.tile([128, LT], F32, name="vp2", tag="tp", bufs=2)
                    for ci in range(2):
                        nc.tensor.matmul(
                            vp2, wva_sb[:, ci, 512 + co * 128:512 + (co + 1) * 128],
                            tok[:, ci, :], start=(ci == 0), stop=(ci == 1))
                    nc.vector.tensor_scalar_add(vv_sb[:, co, n, :], vp2,
                                                bva_sb[:, 4 + co:5 + co])
            # scores: prod_n = kv * qv (broadcast over n), block-diag head sum
            prod = tail.tile([128, 2, N, LT], MMDT)
            for hg in range(2):
                qv_b = bass.AP(tensor=qv_sb.tensor, offset=qv_sb[:, hg, :].offset,
                               ap=[qv_sb[:, hg, :].ap[0], [0, N]]
                               + [qv_sb[:, hg, :].ap[-1]])
                nc.vector.tensor_mul(prod[:, hg, :, :], kv_sb[:, hg, :, :], qv_b)
            p_rep = tail.tile([128, 2, N, LT], MMDT)
            for hg in range(2):
                for n in range(N):
                    srp = ptail.tile([128, LT], F32, name="srp", tag="tp", bufs=2)
                    nc.tensor.matmul(srp, bd_sb, prod[:, hg, n, :],
                                     start=True, stop=True)
                    nc.scalar.activation(out=p_rep[:, hg, n, :], in_=srp,
                                         func=AF.Exp, bias=0.0, scale=1.0)
            # denominators + normalize + AV (values replicated per head)
            ovT = tail.tile([128, 2, LT], MMDT)
            for hg in range(2):
                den = ttmp.tile([128, LT], F32, name="tden", tag="tden", bufs=1)
                nc.vector.tensor_add(den, p_rep[:, hg, 0, :], p_rep[:, hg, 1, :])
                nc.vector.tensor_add(den, den, p_rep[:, hg, 2, :])
                nc.vector.tensor_add(den, den, p_rep[:, hg, 3, :])
                rec = ttmp.tile([128, LT], F32, name="trec", tag="trec", bufs=1)
                nc.vector.reciprocal_approx_fast(rec, den)
                acc = ttmp.tile([128, LT], F32, name="tacc", tag="tacc", bufs=1)
                wv0 = ttmp.tile([128, LT], F32, name="twv", tag="twv", bufs=1)
                nc.vector.tensor_mul(acc, p_rep[:, hg, 0, :], vv_sb[:, hg, 0, :])
                for n in range(1, N):
                    nc.vector.tensor_mul(wv0, p_rep[:, hg, n, :], vv_sb[:, hg, n, :])
                    nc.vector.tensor_add(acc, acc, wv0)
                nc.vector.tensor_mul(ovT[:, hg, :], acc, rec)
            # out-proj -> agg ; op1 ; op-LN ; gelu ; op2 ; + rw*fmean
            agg_sb = tail.tile([128, 2, LT], MMDT)
            h1_sb = tail.tile([128, 2, LT], F32)
            for co in range(2):
                agp = ptail.tile([128, LT], F32, name="agp", tag="tp", bufs=2)
                for ci in range(2):
                    nc.tensor.matmul(agp, wova_sb[:, ci, co * 128:(co + 1) * 128],
                                     ovT[:, ci, :], start=(ci == 0), stop=(ci == 1))
                nc.vector.tensor_scalar_add(agg_sb[:, co, :], agp,
                                            bova_sb[:, co:co + 1])
            for co in range(2):
                h1p = ptail.tile([128, LT], F32, name="h1p", tag="tp", bufs=2)
                for ci in range(2):
                    nc.tensor.matmul(h1p, op1_sb[:, ci, co * 128:(co + 1) * 128],
                                     agg_sb[:, ci, :], start=(ci == 0), stop=(ci == 1))
                nc.vector.tensor_scalar_add(h1_sb[:, co, :], h1p,
                                            bop1_sb[:, co:co + 1])
            lnt_sb = tail.tile([128, 2, LT], F32)
            with tc.tile_pool(name="pstats4", bufs=1, space="PSUM") as pstats4:
                _ln_feature_major(nc, pstats4, ttmp,
                                  [h1_sb[:, 0, :], h1_sb[:, 1, :]],
                                  [olg_sb[:, 0:1], olg_sb[:, 1:2]],
                                  [olb_sb[:, 0:1], olb_sb[:, 1:2]],
                                  [lnt_sb[:, 0, :], lnt_sb[:, 1, :]],
                                  ones_col, ones_row, LT, "opln", eps1)
            g_sb = tail.tile([128, 2, LT], MMDT)
            for co in range(2):
                nc.scalar.activation(out=g_sb[:, co, :], in_=lnt_sb[:, co, :],
                                     func=AF.Gelu, bias=0.0, scale=1.0)
            out_sb = tail.tile([128, 2, LT], F32)
            for co in range(2):
                f2p = ptail.tile([128, LT], F32, name="f2p", tag="tp", bufs=2)
                for ci in range(2):
                    nc.tensor.matmul(f2p, op2_sb[:, ci, co * 128:(co + 1) * 128],
                                     g_sb[:, ci, :], start=(ci == 0), stop=(ci == 1))
                nc.vector.tensor_scalar_mul(out_sb[:, co, :], fm_sb[:, co, :],
                                            float(residual_weight))
                nc.vector.tensor_add(out_sb[:, co, :], out_sb[:, co, :], f2p)
                nc.vector.tensor_scalar_add(out_sb[:, co, :], out_sb[:, co, :],
                                            bop2_sb[:, co:co + 1])
            ov = o_t.rearrange("(a p) t -> p a t", p=128)
            for co in range(2):
                nc.sync.dma_start(out=ov[:, co, :], in_=out_sb[:, co, :])

    nc.finalize()
    return nc


def _emit_avden(nc, pending, av_ps, den_ps, ones32):
    pe, hg, pr, g_kt, vc, kt = pending
    for j in range(2):
        hh = pr * 2 + j
        h = hg * 4 + hh
        nc.tensor.matmul(
            av_ps[hg][hh * 32:(hh + 1) * 32, :],
            vc[:, kt, h * 32:(h + 1) * 32],
            pe[:, j, :],
            start=(g_kt == 0), stop=(g_kt == 31),
            tile_position=(0, hh * 32))
        nc.tensor.matmul(
            den_ps[hg][hh * 32:(hh + 1) * 32, :],
            ones32, pe[:, j, :],
            start=(g_kt == 0), stop=(g_kt == 31),
            tile_position=(0, hh * 32))


_CACHED = {}


def _prep_inputs(inputs):
    """Host-side sharding/layout (numpy transposes + dtype casts only)."""
    f = np.ascontiguousarray(inputs["features"], np.float32)     # [4,1024,256]
    pos = np.asarray(inputs["pos_emb"], np.float32)              # [1024,256]
    ve = np.asarray(inputs["view_emb"], np.float32)              # [16,256]

    def bf(a):
        return np.ascontiguousarray(np.asarray(a, np.float32)).astype(
            ml_dtypes.bfloat16)

    def col_layout(b, k):
        b = np.asarray(b, np.float32)
        return np.ascontiguousarray(
            b.reshape(b.shape[:-1] + (k, 128)).swapaxes(-1, -2))

    base = dict(
        wqkvT=bf(np.asarray(inputs["attn_qkv_w"]).transpose(0, 2, 1)),
        bqkv_c=col_layout(inputs["attn_qkv_b"], 6),
        bv_row=np.ascontiguousarray(np.asarray(inputs["attn_qkv_b"])[:, 512:768]),
        woT=bf(np.asarray(inputs["attn_out_w"]).transpose(0, 2, 1)),
        bo_c=col_layout(inputs["attn_out_b"], 2),
        w1T=bf(np.asarray(inputs["ff1_w"]).transpose(0, 2, 1)),
        b1_c=col_layout(inputs["ff1_b"], 8),
        w2T=bf(np.asarray(inputs["ff2_w"]).transpose(0, 2, 1)),
        b2_c=col_layout(inputs["ff2_b"], 2),
        ln1g=col_layout(inputs["ln1_g"], 2), ln1b=col_layout(inputs["ln1_b"], 2),
        ln2g=col_layout(inputs["ln2_g"], 2), ln2b=col_layout(inputs["ln2_b"], 2),
        flng=col_layout(inputs["fln_g"], 2), flnb=col_layout(inputs["fln_b"], 2),
        wvaT=bf(np.asarray(inputs["va_qkv_w"]).T),
        bva_c=col_layout(inputs["va_qkv_b"], 6),
        bvva_row=np.ascontiguousarray(np.asarray(inputs["va_qkv_b"])[512:768][None, :]),
        wovaT=bf(np.asarray(inputs["va_out_w"]).T),
        bova_c=col_layout(inputs["va_out_b"], 2),
        op1T=bf(np.asarray(inputs["op1_w"]).T),
        bop1_c=col_layout(inputs["op1_b"], 2),
        oplng=col_layout(inputs["op_ln_g"], 2), oplnb=col_layout(inputs["op_ln_b"], 2),
        op2T=bf(np.asarray(inputs["op2_w"]).T),
        bop2_c=col_layout(inputs["op2_b"], 2),
        bd=np.kron(np.eye(4, dtype=np.float32),
                   np.ones((32, 32), np.float32)).astype(ml_dtypes.bfloat16),
    )
    gqT_full = np.ascontiguousarray(np.asarray(inputs["global_query"])[0].T)
    fmeanT_full = np.ascontiguousarray(f.mean(0).T)              # [256, 1024]
    in_maps = []
    for c in range(8):
        m = dict(base)
        lsl = slice(c * LT, (c + 1) * LT)
        # tokens (n, i): token n*LT + i = features[n, c*LT + i]
        fT_c = np.concatenate([f[n, lsl, :].T for n in range(N)], axis=1)
        m["fT"] = np.ascontiguousarray(fT_c)                     # [256, 512]
        posv = np.concatenate(
            [(pos[lsl, :] + ve[n][None, :]).T for n in range(N)], axis=1)
        m["posvT"] = np.ascontiguousarray(posv)                  # [256, 512]
        m["gqT"] = np.ascontiguousarray(gqT_full[:, lsl]).astype(
            ml_dtypes.bfloat16)
        m["fmeanT"] = np.ascontiguousarray(fmeanT_full[:, lsl])
        in_maps.append(m)
    return in_maps


def kernel(**inputs) -> np.ndarray:
    rw = float(np.asarray(inputs["residual_weight"]))
    key = "nc"
    if key not in _CACHED:
        _CACHED[key] = build(rw)
    nc = _CACHED[key]
    in_maps = _prep_inputs(inputs)
    res = run_bass_kernel_spmd(nc, in_maps, core_ids=list(range(8)))
    out = np.concatenate([res.results[c]["o_t"] for c in range(8)], axis=1)
    return np.ascontiguousarray(out.T)[None].astype(np.float32)


if __name__ == "__main__":
    pass


# revision 13
# speedup vs baseline: 1488.8439x; 1488.8439x over previous
"""CrossViewSelfAttentionFusion Trainium2 kernel (8-core SPMD).

Sharding: tokens are sharded by L-position — core R owns positions
l in [R*128, (R+1)*128) of ALL 4 views (512 tokens, ordered n-major).
Encoder self-attention is full attention over all 4096 tokens, so K/V
are AllGathered (bf16) per layer; token order inside the gathered
buffer is irrelevant. The cross-view tail at position l needs exactly
the 4 view-tokens at l, which the core already owns -> tail is fully
local and the output is assembled on the host.

Layout: activations feature-major (x^T: [C partitions, tokens free]).
"""
import math
import numpy as np
from contextlib import ExitStack

import ml_dtypes
import concourse.bass as bass
import concourse.bacc as bacc
import concourse.tile as tile
from concourse import mybir
from concourse.bass_utils import run_bass_kernel_spmd

F32 = mybir.dt.float32
BF16 = mybir.dt.bfloat16
AF = mybir.ActivationFunctionType
ALU = mybir.AluOpType

N, L, C, NH, NL = 4, 1024, 256, 8, 3
DFF = 4 * C
S = N * L            # 4096 tokens
T = S // 8           # 512 tokens per core
LT = L // 8          # 128 L-positions per core
DH = C // NH         # 32
QSCALE = 1.0 / math.sqrt(DH)
EPS = 1e-5
NCH = 8              # gathered key chunks (one per rank)
KT_PER_CH = 4

MMDT = BF16


def _ln_feature_major(nc, pstats, tmp, x_tiles, g_cols, b_cols, out_tiles, ones_col,
                      ones_row, width, tag, eps1=None):
    """LayerNorm over the 256-channel partition axis, feature-major tiles.
    x_tiles: 2 SBUF APs [128, width] (fp32). out_tiles: 2 SBUF APs (MMDT)."""
    sq = tmp.tile([128, 2, width], MMDT, name=f"lnsq_{tag}", tag="lnsq", bufs=1)
    xb = tmp.tile([128, 2, width], MMDT, name=f"lnxb_{tag}", tag="lnxb", bufs=1)
    for i in range(2):
        nc.vector.tensor_mul(sq[:, i, :], x_tiles[i], x_tiles[i])
        nc.vector.tensor_copy(xb[:, i, :], x_tiles[i])
    st0 = pstats.tile([1, width], F32, name=f"lnst0_{tag}", tag="lnst0", bufs=1)
    st1 = pstats.tile([1, width], F32, name=f"lnst1_{tag}", tag="lnst1", bufs=1)
    nc.tensor.matmul(st0, ones_col, xb[:, 0, :], start=True, stop=False)
    nc.tensor.matmul(st0, ones_col, xb[:, 1, :], start=False, stop=True)
    nc.tensor.matmul(st1, ones_col, sq[:, 0, :], start=True, stop=False)
    nc.tensor.matmul(st1, ones_col, sq[:, 1, :], start=False, stop=True)
    mean = tmp.tile([1, width], F32, name=f"lnmean_{tag}", tag="lnmean", bufs=1)
    var = tmp.tile([1, width], F32, name=f"lnvar_{tag}", tag="lnvar", bufs=1)
    rstd = tmp.tile([1, width], F32, name=f"lnrstd_{tag}", tag="lnrstd", bufs=1)
    nmr = tmp.tile([1, width], F32, name=f"lnnmr_{tag}", tag="lnnmr", bufs=1)
    nc.vector.tensor_scalar_mul(mean, st0, 1.0 / C)
    nc.vector.tensor_mul(var, mean, mean)
    nc.vector.tensor_scalar_mul(nmr, st1, 1.0 / C)
    nc.vector.tensor_sub(var, nmr, var)
    # rsqrt(var + eps) = exp(-0.5 * ln(var + eps)); Ln/Exp share one table
    nc.scalar.activation(out=rstd, in_=var, func=AF.Ln, bias=eps1, scale=1.0)
    nc.scalar.activation(out=rstd, in_=rstd, func=AF.Exp, bias=0.0, scale=-0.5)
    nc.vector.tensor_scalar_mul(nmr, mean, -1.0)
    nc.vector.tensor_mul(nmr, nmr, rstd)
    smb0 = tmp.tile([1, width], MMDT, name=f"lnsmb0_{tag}", tag="lnsmb0", bufs=1)
    smb1 = tmp.tile([1, width], MMDT, name=f"lnsmb1_{tag}", tag="lnsmb1", bufs=1)
    nc.vector.tensor_copy(smb0, rstd)
    nc.vector.tensor_copy(smb1, nmr)
    bc = pstats.tile([128, 2, width], F32, name=f"lnbc_{tag}", tag="lnbc", bufs=1)
    nc.tensor.matmul(bc[:, 0, :], ones_row, smb0, start=True, stop=True)
    nc.tensor.matmul(bc[:, 1, :], ones_row, smb1, start=True, stop=True)
    t = tmp.tile([128, 2, width], F32, name=f"lnt_{tag}", tag="lnt", bufs=1)
    for i in range(2):
        nc.vector.tensor_mul(t[:, i, :], x_tiles[i], bc[:, 0, :])
        nc.vector.tensor_add(t[:, i, :], t[:, i, :], bc[:, 1, :])
        nc.vector.tensor_scalar(out=out_tiles[i], in0=t[:, i, :], scalar1=g_cols[i],
                                scalar2=b_cols[i], op0=ALU.mult, op1=ALU.add)


def build(residual_weight: float):
    nc = bacc.Bacc("TRN2", target_bir_lowering=False, debug=False, num_devices=8)

    def inp(name, shape, dt=F32):
        return nc.dram_tensor(name, shape, dt, kind="ExternalInput")

    fT = inp("fT", (256, T))                 # features, token cols (n-major)
    posvT = inp("posvT", (256, T))           # pos + view emb, same order
    wqkvT = inp("wqkvT", (NL, 256, 3 * C), BF16)
    bqkv_c = inp("bqkv_c", (NL, 128, 6))
    bv_row = inp("bv_row", (NL, 256))
    woT = inp("woT", (NL, 256, 256), BF16)
    bo_c = inp("bo_c", (NL, 128, 2))
    w1T = inp("w1T", (NL, 256, DFF), BF16)
    b1_c = inp("b1_c", (NL, 128, 8))
    w2T = inp("w2T", (NL, DFF, 256), BF16)
    b2_c = inp("b2_c", (NL, 128, 2))
    ln1g = inp("ln1g", (NL, 128, 2))
    ln1b = inp("ln1b", (NL, 128, 2))
    ln2g = inp("ln2g", (NL, 128, 2))
    ln2b = inp("ln2b", (NL, 128, 2))
    flng = inp("flng", (128, 2))
    flnb = inp("flnb", (128, 2))
    gqT = inp("gqT", (256, LT), BF16)        # this core's global-query slice
    wvaT = inp("wvaT", (256, 3 * C), BF16)
    bva_c = inp("bva_c", (128, 6))
    bvva_row = inp("bvva_row", (1, 256))
    wovaT = inp("wovaT", (256, 256), BF16)
    bova_c = inp("bova_c", (128, 2))
    op1T = inp("op1T", (256, 256), BF16)
    bop1_c = inp("bop1_c", (128, 2))
    oplng = inp("oplng", (128, 2))
    oplnb = inp("oplnb", (128, 2))
    op2T = inp("op2T", (256, 256), BF16)
    bop2_c = inp("bop2_c", (128, 2))
    fmeanT = inp("fmeanT", (256, LT))        # (1/N)*sum_n features[n], L-slice
    bd = inp("bd", (128, 128), BF16)         # block-diagonal 32x32 ones
    maskc = inp("maskc", (128, 8))           # col ch: 0.0 if ch == rank else 1.0

    o_t = nc.dram_tensor("o_t", (256, LT), F32, kind="ExternalOutput")

    kag_ins = [nc.dram_tensor(f"kag_in{i}", (256, T), BF16, kind="Internal")
               for i in range(2)]
    kag_outs = [nc.dram_tensor(f"kag_out{i}", (8, 256, T), BF16, kind="Internal",
                               addr_space="Shared") for i in range(2)]
    vag_ins = [nc.dram_tensor(f"vag_in{i}", (T, 256), BF16, kind="Internal")
               for i in range(2)]
    vag_outs = [nc.dram_tensor(f"vag_out{i}", (8, T, 256), BF16, kind="Internal",
                               addr_space="Shared") for i in range(2)]
    RG = [list(range(8))]

    with tile.TileContext(nc) as tc, ExitStack() as ctx:
        persist = ctx.enter_context(tc.tile_pool(name="persist", bufs=1))
        enc_ctx = ExitStack()
        wpool = enc_ctx.enter_context(tc.tile_pool(name="wpool", bufs=2))
        act = enc_ctx.enter_context(tc.tile_pool(name="act", bufs=2))
        ppool = enc_ctx.enter_context(tc.tile_pool(name="ppool", bufs=4))
        tmp = enc_ctx.enter_context(tc.tile_pool(name="tmp", bufs=2))

        ones_col = persist.tile([128, 1], MMDT)
        nc.vector.memset(ones_col, 1.0)
        ones_row = persist.tile([1, 128], MMDT)
        nc.vector.memset(ones_row, 1.0)
        ones32 = persist.tile([128, 32], MMDT)
        nc.vector.memset(ones32, 1.0)
        eps1 = persist.tile([1, 1], F32)
        nc.vector.memset(eps1, EPS)
        mask_sb = persist.tile([128, 8], F32)
        nc.sync.dma_start(out=mask_sb, in_=maskc[:])

        # ------- embed: x = f + (pos + view) -------
        ft_sb = tmp.tile([128, 2, T], F32)
        nc.sync.dma_start(out=ft_sb, in_=fT.rearrange("(a p) t -> p a t", p=128))
        pos_sb = tmp.tile([128, 2, T], F32)
        nc.sync.dma_start(out=pos_sb, in_=posvT.rearrange("(a p) t -> p a t", p=128))
        x_sb = persist.tile([128, 2, T], F32)
        for i in range(2):
            nc.vector.tensor_add(x_sb[:, i, :], pos_sb[:, i, :], ft_sb[:, i, :])

        # ================= encoder layers =================
        for layer in range(NL):
            # --- weights (bf16 in DRAM already) ---
            wqkv_sb = wpool.tile([128, 2, 3 * C], MMDT, name="wqkv_sb", tag="wqkv")
            nc.gpsimd.dma_start(out=wqkv_sb,
                                in_=wqkvT[layer].rearrange("(a p) o -> p a o", p=128))
            wo_sb = wpool.tile([128, 2, 256], MMDT, name="wo_sb", tag="wo")
            nc.gpsimd.dma_start(out=wo_sb,
                                in_=woT[layer].rearrange("(a p) o -> p a o", p=128))
            w1_sb = wpool.tile([128, 2, DFF], MMDT, name="w1_sb", tag="w1")
            nc.gpsimd.dma_start(out=w1_sb,
                                in_=w1T[layer].rearrange("(a p) o -> p a o", p=128))
            w2_sb = wpool.tile([128, 8, 256], MMDT, name="w2_sb", tag="w2")
            nc.gpsimd.dma_start(out=w2_sb,
                                in_=w2T[layer].rearrange("(a p) o -> p a o", p=128))
            bq_sb = wpool.tile([128, 6], F32, name="bq_sb", tag="bq")
            nc.sync.dma_start(out=bq_sb, in_=bqkv_c[layer])
            bvr_sb = wpool.tile([1, 256], MMDT, name="bvr_sb", tag="bvr")
            nc.gpsimd.dma_start(out=bvr_sb, in_=bv_row[layer][None, :])
            bo_sb = wpool.tile([128, 2], F32, name="bo_sb", tag="bo")
            nc.sync.dma_start(out=bo_sb, in_=bo_c[layer])
            b1_sb = wpool.tile([128, 8], F32, name="b1_sb", tag="b1")
            nc.sync.dma_start(out=b1_sb, in_=b1_c[layer])
            b2_sb = wpool.tile([128, 2], F32, name="b2_sb", tag="b2")
            nc.sync.dma_start(out=b2_sb, in_=b2_c[layer])
            l1g_sb = wpool.tile([128, 2], F32, name="l1g_sb", tag="l1g")
            nc.sync.dma_start(out=l1g_sb, in_=ln1g[layer])
            l1b_sb = wpool.tile([128, 2], F32, name="l1b_sb", tag="l1b")
            nc.sync.dma_start(out=l1b_sb, in_=ln1b[layer])
            l2g_sb = wpool.tile([128, 2], F32, name="l2g_sb", tag="l2g")
            nc.sync.dma_start(out=l2g_sb, in_=ln2g[layer])
            l2b_sb = wpool.tile([128, 2], F32, name="l2b_sb", tag="l2b")
            nc.sync.dma_start(out=l2b_sb, in_=ln2b[layer])

            # --- LN1 ---
            ln_sb = act.tile([128, 2, T], MMDT, name="ln_sb", tag="ln")
            with tc.tile_pool(name="pstats", bufs=1, space="PSUM") as pstats:
                _ln_feature_major(nc, pstats, tmp,
                                  [x_sb[:, 0, :], x_sb[:, 1, :]],
                                  [l1g_sb[:, 0:1], l1g_sb[:, 1:2]],
                                  [l1b_sb[:, 0:1], l1b_sb[:, 1:2]],
                                  [ln_sb[:, 0, :], ln_sb[:, 1, :]],
                                  ones_col, ones_row, T, f"l{layer}a", eps1)

            # --- QKV (bias/scale on DVE, not Scalar) ---
            q_sb = act.tile([128, 2, T], MMDT, name="q_sb", tag="q")
            k_sb = act.tile([128, 2, T], MMDT, name="k_sb", tag="k")
            v_sb = act.tile([128, 4, 256], MMDT, name="v_sb", tag="v")
            bqs_sb = tmp.tile([128, 2], F32, name="bqs_sb", tag="bqs")
            nc.vector.tensor_scalar_mul(bqs_sb, bq_sb[:, 0:2], QSCALE)
            with tc.tile_pool(name="pqkv", bufs=2, space="PSUM") as pqkv:
                for co in range(2):
                    kp = pqkv.tile([128, T], F32, name="kp", tag="qp", bufs=2)
                    for ci in range(2):
                        nc.tensor.matmul(
                            kp, wqkv_sb[:, ci, 256 + co * 128:256 + (co + 1) * 128],
                            ln_sb[:, ci, :], start=(ci == 0), stop=(ci == 1))
                    nc.vector.tensor_scalar_add(k_sb[:, co, :], kp,
                                                bq_sb[:, 2 + co:3 + co])
                # K AllGather as soon as K is ready (overlaps V/Q compute).
                # Buffers alternate by layer parity to avoid WAR stalls on
                # the previous layer's chunk reads.
                kag_in = kag_ins[layer % 2]
                kag_out = kag_outs[layer % 2]
                vag_in = vag_ins[layer % 2]
                vag_out = vag_outs[layer % 2]
                kag_v = kag_in.rearrange("(a p) t -> p a t", p=128)
                for i in range(2):
                    nc.sync.dma_start(out=kag_v[:, i, :], in_=k_sb[:, i, :])
                nc.gpsimd.collective_compute(
                    "AllGather", ALU.bypass, replica_groups=RG,
                    ins=[kag_in[:].opt()], outs=[kag_out[:].opt()])

                bvb = pqkv.tile([128, 256], F32, name="bvb", tag="bvb")
                nc.tensor.matmul(bvb, ones_row, bvr_sb, start=True, stop=True)
                bvb_sb = tmp.tile([128, 256], F32, name="bvb_sb", tag="bvb_sb")
                nc.vector.tensor_copy(bvb_sb, bvb)
                for tt in range(4):
                    vp = pqkv.tile([128, 256], F32, name="vp", tag="vp", bufs=2)
                    for ci in range(2):
                        nc.tensor.matmul(
                            vp, ln_sb[:, ci, tt * 128:(tt + 1) * 128],
                            wqkv_sb[:, ci, 512:768],
                            start=(ci == 0), stop=(ci == 1))
                    nc.vector.tensor_add(v_sb[:, tt, :], vp, bvb_sb)
                vag_v = vag_in.rearrange("(t p) c -> p t c", p=128)
                for tt in range(4):
                    nc.sync.dma_start(out=vag_v[:, tt, :], in_=v_sb[:, tt, :])
                nc.gpsimd.collective_compute(
                    "AllGather", ALU.bypass, replica_groups=RG,
                    ins=[vag_in[:].opt()], outs=[vag_out[:].opt()])

                for co in range(2):
                    qp = pqkv.tile([128, T], F32, name="qp", tag="qp", bufs=2)
                    for ci in range(2):
                        nc.tensor.matmul(
                            qp, wqkv_sb[:, ci, co * 128:(co + 1) * 128],
                            ln_sb[:, ci, :], start=(ci == 0), stop=(ci == 1))
                    # q = qp*QSCALE + bq*QSCALE  (DVE, per-partition scalars)
                    nc.vector.tensor_scalar(out=q_sb[:, co, :], in0=qp,
                                            scalar1=QSCALE,
                                            scalar2=bqs_sb[:, co:co + 1],
                                            op0=ALU.mult, op1=ALU.add)

            # --- attention ---
            # per (ch, kt, hg): two head-pairs; each pair = 2 score matmuls
            # into one [128, 2, 512] PSUM tile, ONE exp over 1024 free elems,
            # then 2 AV + 2 den matmuls. Score PSUM double-buffered so the
            # PE never waits on the Scalar exp.
            oT_sb = act.tile([128, 2, T], MMDT, name="oT_sb", tag="oT")
            with tc.tile_pool(name="psc", bufs=2, space="PSUM") as psc, \
                 tc.tile_pool(name="pacc", bufs=1, space="PSUM") as pacc:
                av_ps = [pacc.tile([128, T], F32, name=f"av_ps{g}", tag=f"av{g}")
                         for g in range(2)]
                den_ps = [pacc.tile([128, T], F32, name=f"den_ps{g}", tag=f"den{g}")
                          for g in range(2)]
                pending = None  # awaiting AV/den (one step behind the exp)
                NSTEP = (NCH + 1) * KT_PER_CH - 1  # last accumulation step
                # local chunk first (from SBUF, no gather dependency), then
                # the 8 gathered chunks with the local one masked to zero.
                for ci_ch in range(NCH + 1):
                    if ci_ch == 0:
                        kc, vc, dones = k_sb, v_sb, ones32
                    else:
                        ch = ci_ch - 1
                        kc = tmp.tile([128, 2, T], MMDT, name="kc", tag="kc",
                                      bufs=3)
                        nc.gpsimd.dma_start(
                            out=kc,
                            in_=kag_out[ch].rearrange("(a p) t -> p a t", p=128))
                        vcr = tmp.tile([128, 4, 256], MMDT, name="vcr", tag="vcr",
                                       bufs=3)
                        nc.gpsimd.dma_start(
                            out=vcr,
                            in_=vag_out[ch].rearrange("(t p) c -> p t c", p=128))
                        vc = tmp.tile([128, 4, 256], MMDT, name="vcm", tag="vcm",
                                      bufs=3)
                        nc.vector.tensor_scalar_mul(vc, vcr,
                                                    mask_sb[:, ch:ch + 1])
                        dones = tmp.tile([128, 32], MMDT, name="do", tag="do",
                                         bufs=3)
                        nc.vector.tensor_scalar_mul(dones, ones32,
                                                    mask_sb[:, ch:ch + 1])
                    for kt in range(KT_PER_CH):
                        g_kt = ci_ch * KT_PER_CH + kt
                        for hg in range(2):
                            for pr in range(2):
                                sp = psc.tile([128, 2, T], F32, name="sp",
                                              tag="sp", bufs=2)
                                for j in range(2):
                                    hh = pr * 2 + j
                                    nc.tensor.matmul(
                                        sp[:, j, :],
                                        kc[hh * 32:(hh + 1) * 32, hg,
                                           kt * 128:(kt + 1) * 128],
                                        q_sb[hh * 32:(hh + 1) * 32, hg, :],
                                        start=True, stop=True,
                                        tile_position=(hh * 32, 0))
                                pe = ppool.tile([128, 2, T], MMDT, name="pe",
                                                tag="pe", bufs=4)
                                nc.scalar.activation(out=pe, in_=sp, func=AF.Exp,
                                                     bias=0.0, scale=1.0)
                                if pending is not None:
                                    _emit_avden(nc, pending, av_ps, den_ps,
                                                NSTEP)
                                pending = (pe, hg, pr, g_kt, vc, kt, dones)
                _emit_avden(nc, pending, av_ps, den_ps, NSTEP)
                pending = None
                for hg in range(2):
                    rec = tmp.tile([128, T], F32, name="rec", tag="rec", bufs=2)
                    nc.vector.reciprocal_approx_fast(rec, den_ps[hg])
                    nc.vector.tensor_mul(oT_sb[:, hg, :], av_ps[hg], rec)

            # --- out-proj + residual ---
            with tc.tile_pool(name="pproj", bufs=1, space="PSUM") as pproj:
                for co in range(2):
                    app = pproj.tile([128, T], F32, name="app", tag="app", bufs=2)
                    for ci in range(2):
                        nc.tensor.matmul(
                            app, wo_sb[:, ci, co * 128:(co + 1) * 128],
                            oT_sb[:, ci, :], start=(ci == 0), stop=(ci == 1))
                    tres = tmp.tile([128, T], F32, name="tres", tag="tres", bufs=2)
                    nc.vector.tensor_scalar_add(tres, app, bo_sb[:, co:co + 1])
                    nc.vector.tensor_add(x_sb[:, co, :], x_sb[:, co, :], tres)

            # --- LN2 + FFN ---
            ln2_sb = act.tile([128, 2, T], MMDT, name="ln2_sb", tag="ln")
            with tc.tile_pool(name="pstats2", bufs=1, space="PSUM") as pstats2:
                _ln_feature_major(nc, pstats2, tmp,
                                  [x_sb[:, 0, :], x_sb[:, 1, :]],
                                  [l2g_sb[:, 0:1], l2g_sb[:, 1:2]],
                                  [l2b_sb[:, 0:1], l2b_sb[:, 1:2]],
                                  [ln2_sb[:, 0, :], ln2_sb[:, 1, :]],
                                  ones_col, ones_row, T, f"l{layer}b", eps1)
            h_sb = act.tile([128, 8, T], MMDT, name="h_sb", tag="h")
            with tc.tile_pool(name="pffn", bufs=1, space="PSUM") as pffn:
                for fo in range(8):
                    hp = pffn.tile([128, T], F32, name="hp", tag="hp", bufs=4)
                    for ci in range(2):
                        nc.tensor.matmul(
                            hp, w1_sb[:, ci, fo * 128:(fo + 1) * 128],
                            ln2_sb[:, ci, :], start=(ci == 0), stop=(ci == 1))
                    nc.scalar.activation(out=h_sb[:, fo, :], in_=hp, func=AF.Gelu,
                                         bias=b1_sb[:, fo:fo + 1], scale=1.0)
                for co in range(2):
                    fp = pffn.tile([128, T], F32, name="fp", tag="fp", bufs=2)
                    for fo in range(8):
                        nc.tensor.matmul(
                            fp, w2_sb[:, fo, co * 128:(co + 1) * 128],
                            h_sb[:, fo, :], start=(fo == 0), stop=(fo == 7))
                    tres2 = tmp.tile([128, T], F32, name="tres2", tag="tres", bufs=2)
                    nc.vector.tensor_scalar_add(tres2, fp, b2_sb[:, co:co + 1])
                    nc.vector.tensor_add(x_sb[:, co, :], x_sb[:, co, :], tres2)

        # ================= final LN (local) =================
        flg_sb = persist.tile([128, 2], F32)
        nc.sync.dma_start(out=flg_sb, in_=flng[:])
        flb_sb = persist.tile([128, 2], F32)
        nc.sync.dma_start(out=flb_sb, in_=flnb[:])
        fl_sb = act.tile([128, 2, T], MMDT, name="fl_sb", tag="ln")
        with tc.tile_pool(name="pstats3", bufs=1, space="PSUM") as pstats3:
            _ln_feature_major(nc, pstats3, tmp,
                              [x_sb[:, 0, :], x_sb[:, 1, :]],
                              [flg_sb[:, 0:1], flg_sb[:, 1:2]],
                              [flb_sb[:, 0:1], flb_sb[:, 1:2]],
                              [fl_sb[:, 0, :], fl_sb[:, 1, :]],
                              ones_col, ones_row, T, "fl", eps1)
        enc_ctx.close()

        # ============ cross-view tail (local: this core's LT positions) ====
        # fl_sb token order is (n, i): token n*LT + i  <-> view n, position i.
        tail = ctx.enter_context(tc.tile_pool(name="tail", bufs=1))
        ttmp = ctx.enter_context(tc.tile_pool(name="ttmp", bufs=2))
        gq_sb = tail.tile([128, 2, LT], MMDT)
        nc.gpsimd.dma_start(out=gq_sb, in_=gqT.rearrange("(a p) t -> p a t", p=128))
        wva_sb = tail.tile([128, 2, 3 * C], MMDT)
        nc.gpsimd.dma_start(out=wva_sb, in_=wvaT.rearrange("(a p) o -> p a o", p=128))
        bva_sb = tail.tile([128, 6], F32)
        nc.sync.dma_start(out=bva_sb, in_=bva_c[:])
        bvva_sb = tail.tile([1, 256], MMDT)
        nc.gpsimd.dma_start(out=bvva_sb, in_=bvva_row[:])
        wova_sb = tail.tile([128, 2, 256], MMDT)
        nc.gpsimd.dma_start(out=wova_sb, in_=wovaT.rearrange("(a p) o -> p a o", p=128))
        bova_sb = tail.tile([128, 2], F32)
        nc.sync.dma_start(out=bova_sb, in_=bova_c[:])
        op1_sb = tail.tile([128, 2, 256], MMDT)
        nc.gpsimd.dma_start(out=op1_sb, in_=op1T.rearrange("(a p) o -> p a o", p=128))
        bop1_sb = tail.tile([128, 2], F32)
        nc.sync.dma_start(out=bop1_sb, in_=bop1_c[:])
        olg_sb = tail.tile([128, 2], F32)
        nc.sync.dma_start(out=olg_sb, in_=oplng[:])
        olb_sb = tail.tile([128, 2], F32)
        nc.sync.dma_start(out=olb_sb, in_=oplnb[:])
        op2_sb = tail.tile([128, 2, 256], MMDT)
        nc.gpsimd.dma_start(out=op2_sb, in_=op2T.rearrange("(a p) o -> p a o", p=128))
        bop2_sb = tail.tile([128, 2], F32)
        nc.sync.dma_start(out=bop2_sb, in_=bop2_c[:])
        fm_sb = tail.tile([128, 2, LT], F32)
        nc.sync.dma_start(out=fm_sb, in_=fmeanT.rearrange("(a p) t -> p a t", p=128))
        bd_sb = tail.tile([128, 128], MMDT)
        nc.gpsimd.dma_start(out=bd_sb, in_=bd[:])

        # qv/kv/vv projections (feature-major, width LT per view)
        qv_sb = tail.tile([128, 2, LT], MMDT)
        kv_sb = tail.tile([128, 2, N, LT], MMDT)
        vv_sb = tail.tile([128, 2, N, LT], MMDT)
        bqs2 = ttmp.tile([128, 2], F32, name="bqs2", tag="bqs2", bufs=1)
        nc.vector.tensor_scalar_mul(bqs2, bva_sb[:, 0:2], QSCALE)
        with tc.tile_pool(name="ptail", bufs=1, space="PSUM") as ptail:
            for co in range(2):
                qp2 = ptail.tile([128, LT], F32, name="qp2", tag="tp", bufs=2)
                for ci in range(2):
                    nc.tensor.matmul(
                        qp2, wva_sb[:, ci, co * 128:(co + 1) * 128],
                        gq_sb[:, ci, :], start=(ci == 0), stop=(ci == 1))
                nc.vector.tensor_scalar(out=qv_sb[:, co, :], in0=qp2,
                                        scalar1=QSCALE, scalar2=bqs2[:, co:co + 1],
                                        op0=ALU.mult, op1=ALU.add)
                kp4 = ptail.tile([128, N, LT], F32, name="kp4", tag="tp4", bufs=2)
                vp4 = ptail.tile([128, N, LT], F32, name="vp4", tag="tp4", bufs=2)
                for n in range(N):
                    tok = fl_sb[:, :, n * LT:(n + 1) * LT]
                    for ci in range(2):
                        nc.tensor.matmul(
                            kp4[:, n, :],
                            wva_sb[:, ci, 256 + co * 128:256 + (co + 1) * 128],
                            tok[:, ci, :], start=(ci == 0), stop=(ci == 1))
                    for ci in range(2):
                        nc.tensor.matmul(
                            vp4[:, n, :],
                            wva_sb[:, ci, 512 + co * 128:512 + (co + 1) * 128],
                            tok[:, ci, :], start=(ci == 0), stop=(ci == 1))
                nc.vector.tensor_scalar_add(kv_sb[:, co, :, :], kp4,
                                            bva_sb[:, 2 + co:3 + co])
                nc.vector.tensor_scalar_add(vv_sb[:, co, :, :], vp4,
                                            bva_sb[:, 4 + co:5 + co])
            # scores: prod_n = kv * qv (broadcast over n), block-diag head sum
            prod = tail.tile([128, 2, N, LT], MMDT)
            for hg in range(2):
                qv_b = bass.AP(tensor=qv_sb.tensor, offset=qv_sb[:, hg, :].offset,
                               ap=[qv_sb[:, hg, :].ap[0], [0, N]]
                               + [qv_sb[:, hg, :].ap[-1]])
                nc.vector.tensor_mul(prod[:, hg, :, :], kv_sb[:, hg, :, :], qv_b)
            p_rep = tail.tile([128, 2, N, LT], MMDT)
            for hg in range(2):
                srp = ptail.tile([128, N, LT], F32, name="srp", tag="tp4", bufs=2)
                for n in range(N):
                    nc.tensor.matmul(srp[:, n, :], bd_sb, prod[:, hg, n, :],
                                     start=True, stop=True)
                nc.scalar.activation(out=p_rep[:, hg, :, :], in_=srp,
                                     func=AF.Exp, bias=0.0, scale=1.0)
            # denominators + normalize + AV (values replicated per head)
            ovT = tail.tile([128, 2, LT], MMDT)
            den = ttmp.tile([128, 2, LT], F32, name="tden", tag="tden", bufs=1)
            t2 = ttmp.tile([128, 2, LT], F32, name="tden2", tag="tden2", bufs=1)
            nc.vector.tensor_add(den, p_rep[:, :, 0, :], p_rep[:, :, 1, :])
            nc.vector.tensor_add(t2, p_rep[:, :, 2, :], p_rep[:, :, 3, :])
            nc.vector.tensor_add(den, den, t2)
            rec = ttmp.tile([128, 2, LT], F32, name="trec", tag="trec", bufs=1)
            nc.vector.reciprocal_approx_fast(rec, den)
            wv = ttmp.tile([128, 2, N, LT], MMDT, name="twv", tag="twv", bufs=1)
            nc.vector.tensor_mul(wv, p_rep, vv_sb)
            acc = ttmp.tile([128, 2, LT], F32, name="tacc", tag="tacc", bufs=1)
            nc.vector.tensor_add(acc, wv[:, :, 0, :], wv[:, :, 1, :])
            nc.vector.tensor_add(t2, wv[:, :, 2, :], wv[:, :, 3, :])
            nc.vector.tensor_add(acc, acc, t2)
            nc.vector.tensor_mul(ovT, acc, rec)
            # out-proj -> agg ; op1 ; op-LN ; gelu ; op2 ; + rw*fmean
            agg_sb = tail.tile([128, 2, LT], MMDT)
            h1_sb = tail.tile([128, 2, LT], F32)
            for co in range(2):
                agp = ptail.tile([128, LT], F32, name="agp", tag="tp", bufs=2)
                for ci in range(2):
                    nc.tensor.matmul(agp, wova_sb[:, ci, co * 128:(co + 1) * 128],
                                     ovT[:, ci, :], start=(ci == 0), stop=(ci == 1))
                nc.vector.tensor_scalar_add(agg_sb[:, co, :], agp,
                                            bova_sb[:, co:co + 1])
            for co in range(2):
                h1p = ptail.tile([128, LT], F32, name="h1p", tag="tp", bufs=2)
                for ci in range(2):
                    nc.tensor.matmul(h1p, op1_sb[:, ci, co * 128:(co + 1) * 128],
                                     agg_sb[:, ci, :], start=(ci == 0), stop=(ci == 1))
                nc.vector.tensor_scalar_add(h1_sb[:, co, :], h1p,
                                            bop1_sb[:, co:co + 1])
            lnt_sb = tail.tile([128, 2, LT], F32)
            with tc.tile_pool(name="pstats4", bufs=1, space="PSUM") as pstats4:
                _ln_feature_major(nc, pstats4, ttmp,
                                  [h1_sb[:, 0, :], h1_sb[:, 1, :]],
                                  [olg_sb[:, 0:1], olg_sb[:, 1:2]],
                                  [olb_sb[:, 0:1], olb_sb[:, 1:2]],
                                  [lnt_sb[:, 0, :], lnt_sb[:, 1, :]],
                                  ones_col, ones_row, LT, "opln", eps1)
            g_sb = tail.tile([128, 2, LT], MMDT)
            nc.scalar.activation(out=g_sb, in_=lnt_sb, func=AF.Gelu,
                                 bias=0.0, scale=1.0)
            out_sb = tail.tile([128, 2, LT], F32)
            nc.vector.tensor_scalar_mul(out_sb, fm_sb, float(residual_weight))
            for co in range(2):
                f2p = ptail.tile([128, LT], F32, name="f2p", tag="tp", bufs=2)
                for ci in range(2):
                    nc.tensor.matmul(f2p, op2_sb[:, ci, co * 128:(co + 1) * 128],
                                     g_sb[:, ci, :], start=(ci == 0), stop=(ci == 1))
                nc.vector.tensor_add(out_sb[:, co, :], out_sb[:, co, :], f2p)
                nc.vector.tensor_scalar_add(out_sb[:, co, :], out_sb[:, co, :],
                                            bop2_sb[:, co:co + 1])
            ov = o_t.rearrange("(a p) t -> p a t", p=128)
            for co in range(2):
                nc.sync.dma_start(out=ov[:, co, :], in_=out_sb[:, co, :])

    nc.finalize()
    return nc


def _emit_avden(nc, pending, av_ps, den_ps, nstep):
    pe, hg, pr, g_kt, vc, kt, dones = pending
    for j in range(2):
        hh = pr * 2 + j
        h = hg * 4 + hh
        nc.tensor.matmul(
            av_ps[hg][hh * 32:(hh + 1) * 32, :],
            vc[:, kt, h * 32:(h + 1) * 32],
            pe[:, j, :],
            start=(g_kt == 0), stop=(g_kt == nstep),
            tile_position=(0, hh * 32))
        nc.tensor.matmul(
            den_ps[hg][hh * 32:(hh + 1) * 32, :],
            dones, pe[:, j, :],
            start=(g_kt == 0), stop=(g_kt == nstep),
            tile_position=(0, hh * 32))


_CACHED = {}


def _prep_inputs(inputs):
    """Host-side sharding/layout (numpy transposes + dtype casts only)."""
    f = np.ascontiguousarray(inputs["features"], np.float32)     # [4,1024,256]
    pos = np.asarray(inputs["pos_emb"], np.float32)              # [1024,256]
    ve = np.asarray(inputs["view_emb"], np.float32)              # [16,256]

    def bf(a):
        return np.ascontiguousarray(np.asarray(a, np.float32)).astype(
            ml_dtypes.bfloat16)

    def col_layout(b, k):
        b = np.asarray(b, np.float32)
        return np.ascontiguousarray(
            b.reshape(b.shape[:-1] + (k, 128)).swapaxes(-1, -2))

    base = dict(
        wqkvT=bf(np.asarray(inputs["attn_qkv_w"]).transpose(0, 2, 1)),
        bqkv_c=col_layout(inputs["attn_qkv_b"], 6),
        bv_row=np.ascontiguousarray(np.asarray(inputs["attn_qkv_b"])[:, 512:768]),
        woT=bf(np.asarray(inputs["attn_out_w"]).transpose(0, 2, 1)),
        bo_c=col_layout(inputs["attn_out_b"], 2),
        w1T=bf(np.asarray(inputs["ff1_w"]).transpose(0, 2, 1)),
        b1_c=col_layout(inputs["ff1_b"], 8),
        w2T=bf(np.asarray(inputs["ff2_w"]).transpose(0, 2, 1)),
        b2_c=col_layout(inputs["ff2_b"], 2),
        ln1g=col_layout(inputs["ln1_g"], 2), ln1b=col_layout(inputs["ln1_b"], 2),
        ln2g=col_layout(inputs["ln2_g"], 2), ln2b=col_layout(inputs["ln2_b"], 2),
        flng=col_layout(inputs["fln_g"], 2), flnb=col_layout(inputs["fln_b"], 2),
        wvaT=bf(np.asarray(inputs["va_qkv_w"]).T),
        bva_c=col_layout(inputs["va_qkv_b"], 6),
        bvva_row=np.ascontiguousarray(np.asarray(inputs["va_qkv_b"])[512:768][None, :]),
        wovaT=bf(np.asarray(inputs["va_out_w"]).T),
        bova_c=col_layout(inputs["va_out_b"], 2),
        op1T=bf(np.asarray(inputs["op1_w"]).T),
        bop1_c=col_layout(inputs["op1_b"], 2),
        oplng=col_layout(inputs["op_ln_g"], 2), oplnb=col_layout(inputs["op_ln_b"], 2),
        op2T=bf(np.asarray(inputs["op2_w"]).T),
        bop2_c=col_layout(inputs["op2_b"], 2),
        bd=np.kron(np.eye(4, dtype=np.float32),
                   np.ones((32, 32), np.float32)).astype(ml_dtypes.bfloat16),
    )
    gqT_full = np.ascontiguousarray(np.asarray(inputs["global_query"])[0].T)
    fmeanT_full = np.ascontiguousarray(f.mean(0).T)              # [256, 1024]
    in_maps = []
    for c in range(8):
        m = dict(base)
        lsl = slice(c * LT, (c + 1) * LT)
        # tokens (n, i): token n*LT + i = features[n, c*LT + i]
        fT_c = np.concatenate([f[n, lsl, :].T for n in range(N)], axis=1)
        m["fT"] = np.ascontiguousarray(fT_c)                     # [256, 512]
        posv = np.concatenate(
            [(pos[lsl, :] + ve[n][None, :]).T for n in range(N)], axis=1)
        m["posvT"] = np.ascontiguousarray(posv)                  # [256, 512]
        m["gqT"] = np.ascontiguousarray(gqT_full[:, lsl]).astype(
            ml_dtypes.bfloat16)
        m["fmeanT"] = np.ascontiguousarray(fmeanT_full[:, lsl])
        mk = np.ones((128, 8), np.float32)
        mk[:, c] = 0.0
        m["maskc"] = mk
        in_maps.append(m)
    return in_maps


def kernel(**inputs) -> np.ndarray:
    rw = float(np.asarray(inputs["residual_weight"]))
    key = "nc"
    if key not in _CACHED:
        _CACHED[key] = build(rw)
    nc = _CACHED[key]
    in_maps = _prep_inputs(inputs)
    res = run_bass_kernel_spmd(nc, in_maps, core_ids=list(range(8)))
    out = np.concatenate([res.results[c]["o_t"] for c in range(8)], axis=1)
    return np.ascontiguousarray(out.T)[None].astype(np.float32)


if __name__ == "__main__":
    pass
